# revision 1
# baseline (speedup 1.0000x reference)
"""DeepSeek-MLA prefill kernel for 8 Trainium2 NeuronCores.

Sharding: tensor-parallel over heads (2 heads/core), zero collectives.
Per core:
  1. Fold the low-rank q/kv projections into per-core effective weights on the
     TensorEngine: w_eff_T = (w_up_slice @ w_down).T.
  2. qkv feature-major: qkv_T = w_eff_T.T @ x_t, writing q/k nope parts and v
     transposes directly into attention layouts.
  3. Partial RoPE on DVE (q and k packed in one 128-partition pass),
     reassembly via SBUF->SBUF DMA, RMS-norm via sum-of-squares matmuls;
     k scaled in place, q's scale fused into the softmax exp.
  4. Causal attention per (batch, head): S chunks on PE (fp32r), exp on
     ScalarE with fused row-sum accumulation, 128x128 PE transposes of P,
     PV accumulation on PE, denominators applied via DRAM-broadcast multiply.
  5. out_partial = y @ wo_slice.T (token-major).
Host sums the 8 partial outputs (the all-reduce after wo).
"""

import os
import sys

os.environ.setdefault("JAX_PLATFORMS", "axon,cpu")
if "/opt/trn_rl_repo" not in sys.path:
    sys.path.insert(0, "/opt/trn_rl_repo")

import numpy as np

import concourse.bass as bass
import concourse.tile as tile
from concourse import bacc, mybir
from concourse.bass import ts
from concourse.bass_utils import run_bass_kernel_spmd
from concourse.masks import make_identity

B, T, C = 2, 2048, 2048
H = 16
ROPE_DIM, NOPE_DIM, V_DIM = 64, 64, 128
HEAD_DIM = NOPE_DIM + ROPE_DIM
Q_RANK, KV_RANK = 1536, 512
NCORES = 8
HPC = H // NCORES          # 2 heads per core
NT = B * T                 # 4096 tokens
P = 128
KQ, KK, KC = Q_RANK // P, KV_RANK // P, C // P  # 12, 4, 16
TCH = 512                  # apply-phase token chunk
NTC = NT // TCH            # 16
TQT = T // P               # 16 query tiles per batch
EPS128 = float(np.float32(np.finfo(np.float32).eps)) * HEAD_DIM

F32 = mybir.dt.float32
F32R = mybir.dt.float32r
BF16 = mybir.dt.bfloat16
EXP = mybir.ActivationFunctionType.Exp
SQRT = mybir.ActivationFunctionType.Sqrt
SQUARE = mybir.ActivationFunctionType.Square
MULT = mybir.AluOpType.mult
ADD = mybir.AluOpType.add

_CACHE = {}
_last_results = None


def _fold_q(nc, tc, d_down, d_up, weff, ident):
    """weff [P, KC, 256] (f32r) = (up_slice @ down).T; single pass over down."""
    with tc.tile_pool(name="foldq", bufs=1) as fp:
        up_t = fp.tile([P, KQ, 2 * P], F32R, tag="up", name="upq")
        nc.sync.dma_start(up_t[:], d_up.rearrange("(kt p) m -> p kt m", p=P))
        nonT = fp.tile([P, 2, C], F32, tag="nonT", name="nonTq")
        with tc.tile_pool(name="foldqps", bufs=1, space="PSUM") as fps:
            pss = {}
            for k in range(KQ):
                dk = fp.tile([P, C], F32R, tag="downk", name="downkq", bufs=3)
                nc.sync.dma_start(dk[:], d_down[ts(k, P), :])
                for o in range(2):
                    for cc in range(4):
                        if k == 0:
                            pss[(o, cc)] = fps.tile([P, 512], F32, tag=f"f{o}_{cc}",
                                                    name=f"fq{o}_{cc}")
                        nc.tensor.matmul(pss[(o, cc)][:], up_t[:, k, ts(o, P)],
                                         dk[:, ts(cc, 512)],
                                         start=(k == 0), stop=(k == KQ - 1))
            for o in range(2):
                for cc in range(4):
                    nc.any.tensor_copy(nonT[:, o, ts(cc, 512)], pss[(o, cc)][:])
        with tc.tile_pool(name="foldqtr", bufs=2, space="PSUM") as ftr:
            for cc in range(KC):
                pt = ftr.tile([P, 2 * P], F32, tag="ft", name="ftq")
                for o in range(2):
                    nc.tensor.transpose(pt[:, ts(o, P)], nonT[:, o, ts(cc, P)],
                                        ident[:])
                nc.any.tensor_copy(weff[:, cc, :], pt[:])


def _fold_kv(nc, tc, d_down, d_up, weff, ident):
    """weff [P, KC, 512] (f32r); o-outer (4 PSUM tags), down streamed per (o,k),
    per-o transposes so only one [P, C] nonT strip is live."""
    with tc.tile_pool(name="foldk", bufs=1) as fp:
        up_t = fp.tile([P, KK, 4 * P], F32R, tag="up", name="upk")
        nc.sync.dma_start(up_t[:], d_up.rearrange("(kt p) m -> p kt m", p=P))
        for o in range(4):
            nonT = fp.tile([P, C], F32, tag="nonT", name="nonTk", bufs=2)
            with tc.tile_pool(name="foldkps", bufs=1, space="PSUM") as fps, \
                 tc.tile_pool(name="foldktr", bufs=2, space="PSUM") as ftr:
                pss = {}
                for k in range(KK):
                    dk = fp.tile([P, C], F32R, tag="downk", name="downkk", bufs=2)
                    nc.sync.dma_start(dk[:], d_down[ts(k, P), :])
                    for cc in range(4):
                        if k == 0:
                            pss[cc] = fps.tile([P, 512], F32, tag=f"fk{cc}",
                                               name=f"fk{cc}")
                        nc.tensor.matmul(pss[cc][:], up_t[:, k, ts(o, P)],
                                         dk[:, ts(cc, 512)],
                                         start=(k == 0), stop=(k == KK - 1))
                for cc in range(4):
                    nc.any.tensor_copy(nonT[:, ts(cc, 512)], pss[cc][:])
                for cg in range(4):  # transpose groups of 4 C-tiles
                    pt = ftr.tile([P, 512], F32, tag="ft", name="ftk")
                    for u in range(4):
                        nc.tensor.transpose(pt[:, ts(u, P)],
                                            nonT[:, ts(4 * cg + u, P)], ident[:])
                    for u in range(4):
                        nc.any.tensor_copy(weff[:, 4 * cg + u, ts(o, P)],
                                           pt[:, ts(u, P)])


def _build():
    nc = bacc.Bacc("TRN2", target_bir_lowering=False, debug=False,
                   enable_asserts=False, num_devices=NCORES)

    d_xt = nc.dram_tensor("xt", (C, NT), BF16, kind="ExternalInput").ap()
    d_wqd = nc.dram_tensor("wqd", (Q_RANK, C), F32R, kind="ExternalInput").ap()
    d_wqu = nc.dram_tensor("wqu", (Q_RANK, 2 * P), F32R, kind="ExternalInput").ap()
    d_wkd = nc.dram_tensor("wkd", (KV_RANK, C), F32R, kind="ExternalInput").ap()
    d_wku = nc.dram_tensor("wku", (KV_RANK, 4 * P), F32R, kind="ExternalInput").ap()
    d_wot = nc.dram_tensor("wot", (HPC * V_DIM, C), BF16, kind="ExternalInput").ap()
    d_cos = nc.dram_tensor("cos128", (P, NT), BF16, kind="ExternalInput").ap()
    d_sin = nc.dram_tensor("sin128", (P, NT), BF16, kind="ExternalInput").ap()
    d_tri = nc.dram_tensor("tri", (P, P), BF16, kind="ExternalInput").ap()
    d_ones = nc.dram_tensor("ones1", (P, 2), F32R, kind="ExternalInput").ap()
    d_out = nc.dram_tensor("out", (NT, C), F32, kind="ExternalOutput").ap()

    xt_r = d_xt.rearrange("(kt p) t -> p kt t", p=P)

    with tile.TileContext(nc, pool_alloc_mode="queue") as tc:
        with tc.tile_pool(name="small", bufs=1) as sp, \
             tc.tile_pool(name="dram", bufs=1, space="DRAM") as dp:
            ident = sp.tile([P, P], F32, tag="ident", name="ident")
            make_identity(nc, ident[:])
            identb = sp.tile([P, P], BF16, tag="identb", name="identb")
            make_identity(nc, identb[:])
            tri = sp.tile([P, P], BF16, tag="tri", name="tri")
            nc.sync.dma_start(tri[:], d_tri)
            ones1 = sp.tile([P, 2], F32R, tag="ones1", name="ones1")
            nc.sync.dma_start(ones1[:], d_ones)
            epsb = sp.tile([P, 1], F32, tag="epsb", name="epsb")
            nc.gpsimd.memset(epsb[:], EPS128)
            rq = [sp.tile([P, KC * B], F32, tag=f"rq{h}", name=f"rq{h}")
                  for h in range(HPC)]
            wot_t = sp.tile([P, HPC, C], BF16, tag="wot", name="wot")
            nc.sync.dma_start(wot_t[:],
                              d_wot.rearrange("(h p) c -> p h c", p=P))

            with tc.tile_pool(name="attin", bufs=1) as ain:
                qattn = [ain.tile([P, NT], BF16, tag=f"qattn{h}", name=f"qattn{h}")
                         for h in range(HPC)]
                kattn = [ain.tile([P, NT], BF16, tag=f"kattn{h}", name=f"kattn{h}")
                         for h in range(HPC)]
                vtm = [ain.tile([P, KC * B, V_DIM], BF16, tag=f"vtm{h}",
                                name=f"vtm{h}") for h in range(HPC)]

                with tc.tile_pool(name="ropebuf", bufs=1) as rb:
                    # rope operand stacks, 128 partitions:
                    # rows [0:32] q-h0, [32:64] q-h1, [64:96] k-h0, [96:128] k-h1
                    X1 = rb.tile([P, NT], BF16, tag="X1", name="X1")
                    X2 = rb.tile([P, NT], BF16, tag="X2", name="X2")

                    # ---------- kv fold + kv apply ----------
                    with tc.tile_pool(name="weffk", bufs=1) as wkp:
                        weff_k = wkp.tile([P, KC, 4 * P], BF16, tag="weff_k",
                                          name="weff_k")
                        _fold_kv(nc, tc, d_wkd, d_wku, weff_k, ident)
                        with tc.tile_pool(name="apk", bufs=2) as akp, \
                             tc.tile_pool(name="apkps", bufs=1, space="PSUM") as aps, \
                             tc.tile_pool(name="vtps", bufs=2, space="PSUM") as vps:
                            for i in range(NTC):
                                xcs = []
                                for kh in range(2):
                                    xc = akp.tile([P, KC // 2, TCH], BF16,
                                                  tag="xc", name="xck", bufs=4)
                                    nc.gpsimd.dma_start(
                                        xc[:],
                                        xt_r[:, ts(kh, KC // 2), ts(i, TCH)])
                                    xcs.append(xc)
                                for o in range(4):
                                    ps = aps.tile([P, TCH], F32, tag=f"ak{o}",
                                                  name=f"ak{o}")
                                    for k in range(KC):
                                        nc.tensor.matmul(
                                            ps[:], weff_k[:, k, ts(o, P)],
                                            xcs[k // (KC // 2)][:, k % (KC // 2)],
                                            start=(k == 0), stop=(k == KC - 1))
                                    if o == 0:      # k-nope -> kattn[h][0:64]
                                        nc.any.tensor_copy(
                                            kattn[0][0:64, ts(i, TCH)], ps[0:64, :])
                                        nc.any.tensor_copy(
                                            kattn[1][0:64, ts(i, TCH)], ps[64:128, :])
                                    elif o == 1:    # k-rope -> X1/X2 rows 64:128
                                        nc.any.tensor_copy(
                                            X1[64:128, ts(i, TCH)], ps[0:64, :])
                                        nc.any.tensor_copy(
                                            X2[64:128, ts(i, TCH)], ps[64:128, :])
                                    else:           # v head h = o-2
                                        h = o - 2
                                        vstg = akp.tile([P, TCH], BF16, tag="vstg",
                                                        name="vstg")
                                        nc.any.tensor_copy(vstg[:], ps[:])
                                        pv = vps.tile([P, TCH], BF16, tag="vt",
                                                      name="vtp")
                                        for t2 in range(TCH // P):
                                            nc.tensor.transpose(
                                                pv[:, ts(t2, P)],
                                                vstg[:, ts(t2, P)], identb[:])
                                        nc.any.tensor_copy(
                                            vtm[h][:, (TCH // P) * i:(TCH // P) * (i + 1), :]
                                            .rearrange("p a b -> p (a b)"), pv[:])

                    # ---------- q fold + q apply ----------
                    with tc.tile_pool(name="weffq", bufs=1) as wqp:
                        weff_q = wqp.tile([P, KC, 2 * P], BF16, tag="weff_q",
                                          name="weff_q")
                        _fold_q(nc, tc, d_wqd, d_wqu, weff_q, ident)
                        with tc.tile_pool(name="apq", bufs=2) as aqp, \
                             tc.tile_pool(name="apqps", bufs=2, space="PSUM") as aps:
                            for i in range(NTC):
                                xcs = []
                                for kh in range(2):
                                    xc = aqp.tile([P, KC // 2, TCH], BF16,
                                                  tag="xc", name="xcq", bufs=4)
                                    nc.gpsimd.dma_start(
                                        xc[:],
                                        xt_r[:, ts(kh, KC // 2), ts(i, TCH)])
                                    xcs.append(xc)
                                for o in range(2):
                                    ps = aps.tile([P, TCH], F32, tag=f"aq{o}",
                                                  name=f"aq{o}")
                                    for k in range(KC):
                                        nc.tensor.matmul(
                                            ps[:], weff_q[:, k, ts(o, P)],
                                            xcs[k // (KC // 2)][:, k % (KC // 2)],
                                            start=(k == 0), stop=(k == KC - 1))
                                    if o == 0:
                                        nc.any.tensor_copy(
                                            qattn[0][0:64, ts(i, TCH)], ps[0:64, :])
                                        nc.any.tensor_copy(
                                            qattn[1][0:64, ts(i, TCH)], ps[64:128, :])
                                    else:  # q-rope -> X1/X2 rows 0:64
                                        nc.any.tensor_copy(
                                            X1[0:64, ts(i, TCH)], ps[0:64, :])
                                        nc.any.tensor_copy(
                                            X2[0:64, ts(i, TCH)], ps[64:128, :])

                    # ---------- RoPE (q+k in one pass) + reassembly ----------
                    with tc.tile_pool(name="rope2", bufs=1) as rp:
                        cosT = rp.tile([P, NT], BF16, tag="cosT", name="cosT")
                        sinT = rp.tile([P, NT], BF16, tag="sinT", name="sinT")
                        nc.sync.dma_start(cosT[:], d_cos)
                        nc.sync.dma_start(sinT[:], d_sin)
                        lo = rp.tile([P, NT], BF16, tag="lo", name="lo")
                        hib = rp.tile([P, NT], BF16, tag="hib", name="hib")
                        tmp = rp.tile([P, NT], BF16, tag="tmp", name="tmp")
                        for bh in range(B):
                            s = ts(bh, T)
                            nc.vector.tensor_tensor(lo[:, s], X1[:, s], cosT[:, s], MULT)
                            nc.vector.tensor_tensor(tmp[:, s], X2[:, s], sinT[:, s], MULT)
                            nc.vector.tensor_add(lo[:, s], lo[:, s], tmp[:, s])
                            nc.vector.tensor_tensor(hib[:, s], X2[:, s], cosT[:, s], MULT)
                            nc.vector.tensor_tensor(tmp[:, s], X1[:, s], sinT[:, s], MULT)
                            nc.vector.tensor_sub(hib[:, s], hib[:, s], tmp[:, s])
                            for qi in (1, 0):
                                att = qattn if qi == 0 else kattn
                                for h in range(HPC):
                                    r0 = qi * 64 + h * 32
                                    nc.sync.dma_start(att[h][64:96, s],
                                                      lo[r0:r0 + 32, s])
                                    nc.sync.dma_start(att[h][96:128, s],
                                                      hib[r0:r0 + 32, s])

                # ---------- RMS-norm scales ----------
                with tc.tile_pool(name="norm", bufs=1) as npool, \
                     tc.tile_pool(name="normps", bufs=2, space="PSUM") as nps:
                    # k-side per (batch, head): unblocks attention earliest
                    for bn in range(B):
                        for h in range(HPC):
                            sq = npool.tile([P, T], F32R, tag="sq", name="sq",
                                            bufs=2)
                            nc.scalar.activation(sq[:], kattn[h][:, ts(bn, T)],
                                                 SQUARE)
                            ps = nps.tile([P, 2 * TQT], F32, tag="ssq",
                                          name="ssq")
                            for g in range(TQT):
                                nc.tensor.matmul(ps[:, 2 * g:2 * g + 2],
                                                 sq[:, ts(g, P)], ones1[:])
                            rt = npool.tile([P, TQT], F32, tag="rt", name="rt")
                            nc.scalar.activation(
                                rt[:],
                                ps[:].rearrange("p (g two) -> p g two", two=2)[:, :, 0],
                                SQRT, bias=epsb[:])
                            rk = npool.tile([P, TQT], F32, tag="rk", name="rk")
                            nc.vector.reciprocal(rk[:], rt[:])
                            nc.vector.tensor_scalar_mul(
                                rk[:], rk[:], float(np.sqrt(128.0)))
                            scr = dp.tile([1, T], F32, tag=f"rk_scr{bn}_{h}",
                                          name=f"rk_scr{bn}_{h}")
                            nc.sync.dma_start(
                                scr[:].rearrange("o (g p) -> o p g", p=P)[0],
                                rk[:])
                            rkb = npool.tile([P, T], F32, tag="rkb",
                                             name="rkb", bufs=2)
                            nc.sync.dma_start(
                                rkb[:], scr[0:1, :].to_broadcast((P, T)))
                            nc.vector.tensor_tensor(
                                kattn[h][:, ts(bn, T)],
                                kattn[h][:, ts(bn, T)], rkb[:], MULT)
                    # q-side scales
                    for h in range(HPC):
                        sqq = npool.tile([P, NT], F32R, tag="sqq", name="sqq",
                                         bufs=2)
                        nc.scalar.activation(sqq[:], qattn[h][:], SQUARE)
                        ps = nps.tile([P, 2 * KC * B], F32, tag="ssqq",
                                      name="ssqq")
                        for g in range(KC * B):
                            nc.tensor.matmul(ps[:, 2 * g:2 * g + 2],
                                             sqq[:, ts(g, P)], ones1[:])
                        rt = npool.tile([P, KC * B], F32, tag="rt2", name="rt2")
                        nc.scalar.activation(
                            rt[:],
                            ps[:].rearrange("p (g two) -> p g two", two=2)[:, :, 0],
                            SQRT, bias=epsb[:])
                        nc.vector.reciprocal(rq[h][:], rt[:])

                # ---------- causal attention ----------
                yts = {}
                with tc.tile_pool(name="attw", bufs=1) as aw, \
                     tc.tile_pool(name="wo", bufs=2) as wop, \
                     tc.tile_pool(name="sps", bufs=3, space="PSUM") as sps, \
                     tc.tile_pool(name="tps", bufs=2, space="PSUM") as tps, \
                     tc.tile_pool(name="yps", bufs=1, space="PSUM") as yps, \
                     tc.tile_pool(name="wops", bufs=2, space="PSUM") as wps:
                    for b in range(B):
                        for h in range(HPC):
                            q_bh = qattn[h][:, ts(b, T)]
                            k_bh = kattn[h][:, ts(b, T)]
                            yt = aw.tile([P, T], BF16, tag=f"yt{b}_{h}",
                                         name=f"yt{b}_{h}", bufs=1)
                            yts[(b, h)] = yt
                            denoms = aw.tile([P, TQT], F32, tag="denoms",
                                             name="denoms", bufs=4)
                            for cch in range(TQT // 4):
                                pxs = {}
                                for xl in range(4):
                                    x = 4 * cch + xl
                                    wfull = (x + 1) * P
                                    px = aw.tile([P, T], BF16, tag=f"px{xl}",
                                                 name=f"px{xl}", bufs=2)
                                    pxs[x] = px
                                    s_ap = rq[h][:, b * TQT + x:b * TQT + x + 1]
                                    dparts = aw.tile([P, 8], F32, tag="dparts",
                                                     name="dparts", bufs=8)
                                    npart = 0
                                    for off in range(0, wfull, 512):
                                        wc = min(512, wfull - off)
                                        ps = sps.tile([P, 512], F32, tag="s",
                                                      name="sps_t")
                                        nc.tensor.matmul(ps[:, :wc],
                                                         q_bh[:, ts(x, P)],
                                                         k_bh[:, off:off + wc])
                                        last = off + wc == wfull
                                        if not last:
                                            nc.scalar.activation(
                                                px[:, off:off + wc],
                                                ps[:, :wc], EXP, scale=s_ap,
                                                accum_out=dparts[:, npart:npart + 1])
                                            npart += 1
                                        else:
                                            if wc > P:
                                                nc.scalar.activation(
                                                    px[:, off:off + wc - P],
                                                    ps[:, :wc - P], EXP,
                                                    scale=s_ap,
                                                    accum_out=dparts[:, npart:npart + 1])
                                                npart += 1
                                            nc.scalar.activation(
                                                px[:, wfull - P:wfull],
                                                ps[:, wc - P:wc], EXP,
                                                scale=s_ap)
                                            nc.vector.tensor_tensor(
                                                px[:, wfull - P:wfull],
                                                px[:, wfull - P:wfull],
                                                tri[:], MULT)
                                            nc.vector.reduce_sum(
                                                dparts[:, npart:npart + 1],
                                                px[:, wfull - P:wfull],
                                                mybir.AxisListType.X)
                                            npart += 1
                                    nc.vector.reduce_sum(
                                        denoms[:, x:x + 1],
                                        dparts[:, 0:npart],
                                        mybir.AxisListType.X)
                                ptiles = []
                                for j in range(4 * cch + 4):
                                    xl0 = max(0, j - 4 * cch)
                                    pst = tps.tile([P, 512], BF16, tag="tr",
                                                   name="trp")
                                    for xl in range(xl0, 4):
                                        nc.tensor.transpose(
                                            pst[:, ts(xl, P)],
                                            pxs[4 * cch + xl][:, ts(j, P)],
                                            identb[:])
                                    pt_t = aw.tile([P, 512], BF16, tag="pt",
                                                   name="pt_t", bufs=20)
                                    nc.vector.tensor_copy(pt_t[:, xl0 * P:],
                                                          pst[:, xl0 * P:])
                                    ptiles.append((pt_t, xl0 * P))
                                yp = yps.tile([P, 512], F32, tag="y", name="yp")
                                nj = 4 * cch + 4
                                for j in range(nj):
                                    pt_t, off = ptiles[j]
                                    nc.tensor.matmul(
                                        yp[:, off:], vtm[h][:, b * TQT + j, :],
                                        pt_t[:, off:],
                                        start=(j == 0), stop=(j == nj - 1))
                                nc.any.tensor_copy(yt[:, ts(cch, 512)], yp[:])
                            rd = aw.tile([P, TQT], F32, tag="rd", name="rd",
                                         bufs=2)
                            nc.vector.reciprocal(rd[:], denoms[:])
                            scd = dp.tile([1, T], F32, tag=f"rd_scr{b}_{h}",
                                          name=f"rd_scr{b}_{h}")
                            nc.sync.dma_start(
                                scd[:].rearrange("o (g p) -> o p g", p=P)[0],
                                rd[:])
                            rdb = aw.tile([P, T], F32, tag="rdb", name="rdb")
                            nc.sync.dma_start(
                                rdb[:], scd[0:1, :].to_broadcast((P, T)))
                            nc.vector.tensor_tensor(
                                yt[:], yt[:], rdb[:], MULT)
                        if h == HPC - 1:
                          # output projection for this batch (overlaps next batch)
                          for mi in range(TQT):
                            m = b * TQT + mi
                            stg = wop.tile([P, C], F32, tag="stg", name="stg")
                            for n in range(C // 512):
                                ps = wps.tile([P, 512], F32, tag="wo",
                                              name="wops_t")
                                for h2 in range(HPC):
                                    nc.tensor.matmul(
                                        ps[:], yts[(b, h2)][:, ts(mi, P)],
                                        wot_t[:, h2, ts(n, 512)],
                                        start=(h2 == 0), stop=(h2 == HPC - 1))
                                nc.any.tensor_copy(stg[:, ts(n, 512)], ps[:])
                            nc.sync.dma_start(d_out[ts(m, P), :], stg[:])

    nc.compile()
    return nc


def _host_prep(x, cos, sin, wq_down, wq_up, wkv_down, wkv_up, wo):
    import ml_dtypes
    x_t = np.ascontiguousarray(
        np.asarray(x, dtype=np.float32).reshape(NT, C).T
        .astype(ml_dtypes.bfloat16))                               # [C, NT] bf16
    cos_t = np.asarray(cos, dtype=np.float32)[0, :, 0, :].T        # [32, T]
    sin_t = np.asarray(sin, dtype=np.float32)[0, :, 0, :].T
    cos128 = np.ascontiguousarray(np.tile(np.tile(cos_t, (4, 1)), (1, B)).astype(ml_dtypes.bfloat16))
    sin128 = np.ascontiguousarray(np.tile(np.tile(sin_t, (4, 1)), (1, B)).astype(ml_dtypes.bfloat16))
    tri = np.tril(np.ones((P, P))).astype(ml_dtypes.bfloat16)
    ones1 = np.ones((P, 2), dtype=np.float32)

    wq_up = np.asarray(wq_up, dtype=np.float32)
    wkv_up = np.asarray(wkv_up, dtype=np.float32)
    wo = np.asarray(wo, dtype=np.float32)
    wq_down = np.ascontiguousarray(np.asarray(wq_down, dtype=np.float32))
    wkv_down = np.ascontiguousarray(np.asarray(wkv_down, dtype=np.float32))

    in_maps = []
    for core in range(NCORES):
        h0, h1 = HPC * core, HPC * core + 1
        qrows = ([h0 * HEAD_DIM + d for d in range(64)]
                 + [h1 * HEAD_DIM + d for d in range(64)]
                 + [h0 * HEAD_DIM + 64 + d for d in range(32)]
                 + [h1 * HEAD_DIM + 64 + d for d in range(32)]
                 + [h0 * HEAD_DIM + 96 + d for d in range(32)]
                 + [h1 * HEAD_DIM + 96 + d for d in range(32)])
        KVD = HEAD_DIM + V_DIM
        krows = ([h0 * KVD + d for d in range(64)]
                 + [h1 * KVD + d for d in range(64)]
                 + [h0 * KVD + 64 + d for d in range(32)]
                 + [h1 * KVD + 64 + d for d in range(32)]
                 + [h0 * KVD + 96 + d for d in range(32)]
                 + [h1 * KVD + 96 + d for d in range(32)]
                 + [h0 * KVD + HEAD_DIM + d for d in range(V_DIM)]
                 + [h1 * KVD + HEAD_DIM + d for d in range(V_DIM)])
        wqu_slice = np.ascontiguousarray(wq_up[qrows, :].T)        # [1536, 256]
        wku_slice = np.ascontiguousarray(wkv_up[krows, :].T)       # [512, 512]
        ocols = ([h0 * V_DIM + d for d in range(V_DIM)]
                 + [h1 * V_DIM + d for d in range(V_DIM)])
        wot_slice = np.ascontiguousarray(wo[:, ocols].T.astype(ml_dtypes.bfloat16))
        in_maps.append({
            "xt": x_t, "wqd": wq_down, "wqu": wqu_slice,
            "wkd": wkv_down, "wku": wku_slice, "wot": wot_slice,
            "cos128": cos128, "sin128": sin128, "tri": tri, "ones1": ones1,
        })
    return in_maps


def kernel(x, cos, sin, wq_down, wq_up, wkv_down, wkv_up, wo):
    global _last_results
    if "nc" not in _CACHE:
        _CACHE["nc"] = _build()
    nc = _CACHE["nc"]
    in_maps = _host_prep(x, cos, sin, wq_down, wq_up, wkv_down, wkv_up, wo)
    res = run_bass_kernel_spmd(nc, in_maps, core_ids=list(range(NCORES)))
    _last_results = res
    acc = res.results[0]["out"].astype(np.float32)
    for corer in res.results[1:]:
        acc = acc + corer["out"]
    return acc.reshape(B, T, C)

